# revision 6
# baseline (speedup 1.0000x reference)
"""Bayesian linear layer v7: all-bf16 pipeline, overlapped setup, 8 trn2 cores.

y[b,o] = sum_i x[b,i] * (mu[o,i] + softplus(rho[o,i]) * eps_w[b,o,i])
         + bias_mu[o] + softplus(bias_rho[o]) * eps_b[b,o]

Per-core (BL=32 samples, 134 MiB fp32 eps read at ~400 GB/s -> ~340 us):
  - all big loads are gpsimd cast-DMAs fp32->bf16; eps uses the (p c)
    layout (partition p holds o-rows 8p..8p+7 -> 32 KB contiguous reads).
    Q7 queue order: rho, x, eps prefetch, biases, [loop eps], mu at b==8.
  - softplus(rho) computed in natural layout (ACT in-place), then bf16 PE
    transposes (56 ns/tile) -> PSUM bf16 -> DVE evictions.
  - main loop: PE transpose chunks, DVE eviction-fold with sigT (2x mode),
    PE reduce-matmul with x column (software-pipelined one chunk deep).
  - finalize per sample: ACT un-permutes y2 into a yrow tile; a sync-queue
    SBUF->SBUF DMA lands it in Y2[b]. Final Y2 += y_mu + bias; one store.
"""

import numpy as np

import concourse.bass as bass
from concourse import bacc
import concourse.mybir as mybir
import concourse.tile as tile
from concourse.bass import ts
from concourse.bass_utils import run_bass_kernel_spmd
from concourse.masks import make_identity

FP32 = mybir.dt.float32
BF16 = mybir.dt.bfloat16
AF = mybir.ActivationFunctionType

F = 1024
N_CORES = 8
NCH = F // 128
CPP = F // 128  # o-rows per partition in the (p c) layout


def build_nc(BL: int, eps_bufs=7, pt_bufs=4, u_bufs=3, mu_at=8, mu_load_at=2) -> bass.Bass:
    nc = bacc.Bacc(None, target_bir_lowering=False)

    x_d = nc.declare_dram_parameter("x", [BL, F], FP32, isOutput=False)
    mu_d = nc.declare_dram_parameter("weight_mu", [F, F], FP32, isOutput=False)
    rho_d = nc.declare_dram_parameter("weight_rho", [F, F], FP32, isOutput=False)
    bmu_d = nc.declare_dram_parameter("bias_mu", [F], FP32, isOutput=False)
    brho_d = nc.declare_dram_parameter("bias_rho", [F], FP32, isOutput=False)
    epsw_d = nc.declare_dram_parameter("eps_w", [BL, F, F], FP32, isOutput=False)
    epsb_d = nc.declare_dram_parameter("eps_b", [BL, F], FP32, isOutput=False)
    y_d = nc.declare_dram_parameter("y", [BL, F], FP32, isOutput=True)

    epsw_t = epsw_d[:].rearrange("b (p c) i -> b p c i", p=128)
    rho_t = rho_d[:].rearrange("(p c) i -> p c i", p=128)  # same interleave as eps
    mu_t = mu_d[:].rearrange("(c p) i -> p c i", p=128)    # natural o for y_mu

    with tile.TileContext(nc) as tc:
        with (
            tc.tile_pool(name="persist", bufs=1) as persist,
            tc.tile_pool(name="eps", bufs=eps_bufs) as epsp,
            tc.tile_pool(name="setupb", bufs=1) as setbp,
            tc.tile_pool(name="u", bufs=u_bufs) as up,
            tc.tile_pool(name="yrow", bufs=2) as yrowp,
            tc.tile_pool(name="pt", bufs=pt_bufs, space="PSUM") as ptp,
            tc.tile_pool(name="py2", bufs=2, space="PSUM") as py2p,
        ):
            # ---- tiny bias loads first (land in ~2us, unblock the bias chain)
            bmu_b = persist.tile([BL, F], FP32)
            nc.gpsimd.dma_start(
                out=bmu_b, in_=bass.AP(tensor=bmu_d, offset=0, ap=[[0, BL], [1, F]])
            )
            srho_b = persist.tile([BL, F], FP32)
            nc.gpsimd.dma_start(
                out=srho_b, in_=bass.AP(tensor=brho_d, offset=0, ap=[[0, BL], [1, F]])
            )
            epsb_s = persist.tile([BL, F], FP32)
            nc.sync.dma_start(out=epsb_s, in_=epsb_d[:])

            # ---- Q7 head loads: rho (gates sigT), x (gates reduce MMs) ------
            rho_sb = setbp.tile([128, NCH, F], BF16, tag="w16")
            nc.gpsimd.dma_start(out=rho_sb, in_=rho_t)
            xb_nat = persist.tile([BL, F], BF16)
            nc.gpsimd.dma_start(out=xb_nat, in_=x_d[:])

            # ---- eps prefetch: stream starts right behind rho ---------------
            eps_tiles = {}
            PF = min(eps_bufs, BL)
            for b in range(PF):
                eb = epsp.tile([128, NCH, F], BF16, tag="epst")
                nc.gpsimd.dma_start(out=eb, in_=epsw_t[b])
                eps_tiles[b] = eb

            # ---- head compute: ident, xT, softplus(rho) -> sigT -------------
            ident = persist.tile([128, 128], FP32)
            make_identity(nc, ident)
            identb = persist.tile([128, 128], BF16)
            nc.vector.tensor_copy(identb, ident)

            xT = persist.tile([128, NCH, BL], BF16)
            for k in range(NCH):
                ptx = ptp.tile([128, BL], BF16, tag="pt_k")
                nc.tensor.transpose(
                    out=ptx, in_=xb_nat[:, ts(k, 128)], identity=identb[:BL, :BL]
                )
                nc.vector.tensor_copy(xT[:, k, :], ptx)

            # softplus(x) = ln(1 + exp(x)) in natural layout, in place;
            # split by i-halves so k<4 transposes can start after the first.
            for hh in range(2):
                sl = rho_sb[:, :, ts(hh, 512)]
                nc.scalar.activation(out=sl, in_=sl, func=AF.Exp)
                nc.scalar.activation(out=sl, in_=sl, func=AF.Ln, bias=1.0)

            # sigT[i-part, k, col=(c,j)] = softplus(rho[o=8j+c, i]) transposed
            sigT = persist.tile([128, NCH, F], BF16)
            for k in range(NCH):
                pt_k = ptp.tile([128, F], BF16, tag="pt_k")
                for c in range(NCH):
                    nc.tensor.transpose(
                        out=pt_k[:, ts(c, 128)],
                        in_=rho_sb[:, c, ts(k, 128)],
                        identity=identb,
                    )
                nc.vector.tensor_copy(sigT[:, k, :], pt_k)

            # bias part: C0 = bias_mu + softplus(bias_rho) * eps_b
            C0 = persist.tile([BL, F], FP32)
            nc.scalar.activation(out=srho_b, in_=srho_b, func=AF.Exp)
            nc.scalar.activation(out=srho_b, in_=srho_b, func=AF.Ln, bias=1.0)
            nc.vector.tensor_mul(C0, epsb_s, srho_b)
            nc.vector.tensor_add(C0, C0, bmu_b)

            # persistent buffers
            Y2 = persist.tile([BL, F], FP32)
            ymuS = persist.tile([BL, F], FP32)
            muT = persist.tile([128, NCH, F], BF16)

            # ---------------- main loop over samples ----------------
            y2_of = {}
            pending = None  # (b, k, u_k)

            def flush_pending():
                nonlocal pending
                if pending is None:
                    return
                pb, pk, pu = pending
                pending = None
                for h in range(2):
                    nc.tensor.matmul(
                        out=y2_of[pb][h],
                        lhsT=xT[:, pk, pb : pb + 1],
                        rhs=pu[:, ts(h, 512)],
                        start=(pk == 0),
                        stop=(pk == NCH - 1),
                    )
                if pk == NCH - 1:
                    finalize(pb)

            def finalize(pb):
                # y2[h] col = (c - 4h)*128 + j  ->  yrow pos 8j + c
                y2 = y2_of.pop(pb)
                yrow = yrowp.tile([1, F], FP32)
                yrow_jc = yrow.rearrange("r (j c) -> r j c", j=128, c=CPP)
                for h in range(2):
                    nc.scalar.copy(
                        out=yrow_jc[:, :, ts(h, CPP // 2)],
                        in_=y2[h].rearrange("r (c j) -> r j c", c=CPP // 2, j=128),
                    )
                nc.sync.dma_start(out=Y2[pb : pb + 1, :], in_=yrow)

            mu_sb_box = []

            def mu_load():
                mu_sb = setbp.tile([128, NCH, F], BF16, tag="w16")
                nc.gpsimd.dma_start(out=mu_sb, in_=mu_t)
                mu_sb_box.append(mu_sb)

            def mu_setup():
                mu_sb = mu_sb_box.pop()
                for k in range(NCH):
                    pt_k = ptp.tile([128, F], BF16, tag="pt_k")
                    for c in range(NCH):
                        nc.tensor.transpose(
                            out=pt_k[:, ts(c, 128)],
                            in_=mu_sb[:, c, ts(k, 128)],
                            identity=identb,
                        )
                    nc.vector.tensor_copy(muT[:, k, :], pt_k)
                for h in range(2):
                    yp = ptp.tile([BL, 512], FP32, tag="pt_k")
                    for k in range(NCH):
                        nc.tensor.matmul(
                            out=yp,
                            lhsT=xT[:, k, :],
                            rhs=muT[:, k, ts(h, 512)],
                            start=(k == 0),
                            stop=(k == NCH - 1),
                        )
                    nc.vector.tensor_add(ymuS[:, ts(h, 512)], yp, C0[:, ts(h, 512)])

            mu_b = min(mu_at, BL - 1)
            mu_lb = min(mu_load_at, mu_b)
            for b in range(BL):
                if b == mu_lb:
                    mu_load()
                if b == mu_b:
                    flush_pending()
                    mu_setup()

                eb = eps_tiles.pop(b)
                if b + PF < BL:
                    nb = epsp.tile([128, NCH, F], BF16, tag="epst")
                    nc.gpsimd.dma_start(out=nb, in_=epsw_t[b + PF])
                    eps_tiles[b + PF] = nb

                y2_of[b] = [
                    py2p.tile([1, 512], FP32, tag=f"y2_{h}", name=f"y2_{b}_{h}")
                    for h in range(2)
                ]
                for k in range(NCH):
                    pt_k = ptp.tile([128, F], BF16, tag="pt_k")
                    for c in range(NCH):
                        nc.tensor.transpose(
                            out=pt_k[:, ts(c, 128)],
                            in_=eb[:, c, ts(k, 128)],
                            identity=identb,
                        )
                    flush_pending()
                    u_k = up.tile([128, F], BF16)
                    nc.vector.tensor_mul(u_k, pt_k, sigT[:, k, :])
                    pending = (b, k, u_k)

            flush_pending()

            # -------- final combine: y = Y2 + (y_mu + bias) ------------------
            nc.vector.tensor_add(Y2, Y2, ymuS)
            nc.sync.dma_start(out=y_d[:], in_=Y2)

    nc.compile()
    return nc


_NC_CACHE: dict[int, bass.Bass] = {}


def _get_nc(BL: int) -> bass.Bass:
    if BL not in _NC_CACHE:
        _NC_CACHE[BL] = build_nc(BL)
    return _NC_CACHE[BL]


def kernel(x, weight_mu, weight_rho, bias_mu, bias_rho, eps_w, eps_b):
    B = x.shape[0]
    BL = B // N_CORES
    nc = _get_nc(BL)

    x = np.ascontiguousarray(np.asarray(x, dtype=np.float32))
    weight_mu = np.ascontiguousarray(np.asarray(weight_mu, dtype=np.float32))
    weight_rho = np.ascontiguousarray(np.asarray(weight_rho, dtype=np.float32))
    bias_mu = np.ascontiguousarray(np.asarray(bias_mu, dtype=np.float32))
    bias_rho = np.ascontiguousarray(np.asarray(bias_rho, dtype=np.float32))
    eps_w = np.ascontiguousarray(np.asarray(eps_w, dtype=np.float32))
    eps_b = np.ascontiguousarray(np.asarray(eps_b, dtype=np.float32))

    in_maps = []
    for i in range(N_CORES):
        sl = slice(i * BL, (i + 1) * BL)
        in_maps.append(
            {
                "x": x[sl],
                "weight_mu": weight_mu,
                "weight_rho": weight_rho,
                "bias_mu": bias_mu,
                "bias_rho": bias_rho,
                "eps_w": eps_w[sl],
                "eps_b": eps_b[sl],
            }
        )

    res = run_bass_kernel_spmd(nc, in_maps, core_ids=list(range(N_CORES)))
    return np.concatenate([r["y"] for r in res.results], axis=0)


# revision 7
# speedup vs baseline: 1.0149x; 1.0149x over previous
"""Bayesian linear layer v6: all-bf16 pipeline, overlapped setup, 8 trn2 cores.

y[b,o] = sum_i x[b,i] * (mu[o,i] + softplus(rho[o,i]) * eps_w[b,o,i])
         + bias_mu[o] + softplus(bias_rho[o]) * eps_b[b,o]

Per-core (BL=32 samples, 134 MiB fp32 eps read at ~400 GB/s -> ~340 us):
  - all big loads are gpsimd cast-DMAs fp32->bf16; eps uses the (p c)
    layout (partition p holds o-rows 8p..8p+7 -> 32 KB contiguous reads).
    Q7 queue order: rho, x, eps prefetch, biases, [loop eps], mu at b==8.
  - softplus(rho) computed in natural layout (ACT in-place), then bf16 PE
    transposes (56 ns/tile) -> PSUM bf16 -> DVE evictions.
  - main loop: PE transpose chunks, DVE eviction-fold with sigT (2x mode),
    PE reduce-matmul with x column (software-pipelined one chunk deep).
  - finalize per sample: ACT un-permutes y2 into a yrow tile; a sync-queue
    SBUF->SBUF DMA lands it in Y2[b]. Final Y2 += y_mu + bias; one store.
"""

import numpy as np

import concourse.bass as bass
from concourse import bacc
import concourse.mybir as mybir
import concourse.tile as tile
from concourse.bass import ts
from concourse.bass_utils import run_bass_kernel_spmd
from concourse.masks import make_identity

FP32 = mybir.dt.float32
BF16 = mybir.dt.bfloat16
AF = mybir.ActivationFunctionType

F = 1024
N_CORES = 8
NCH = F // 128
CPP = F // 128  # o-rows per partition in the (p c) layout


def build_nc(BL: int, eps_bufs=7, pt_bufs=4, u_bufs=3, mu_at=8) -> bass.Bass:
    nc = bacc.Bacc(None, target_bir_lowering=False)

    x_d = nc.declare_dram_parameter("x", [BL, F], FP32, isOutput=False)
    mu_d = nc.declare_dram_parameter("weight_mu", [F, F], FP32, isOutput=False)
    rho_d = nc.declare_dram_parameter("weight_rho", [F, F], FP32, isOutput=False)
    bmu_d = nc.declare_dram_parameter("bias_mu", [F], FP32, isOutput=False)
    brho_d = nc.declare_dram_parameter("bias_rho", [F], FP32, isOutput=False)
    epsw_d = nc.declare_dram_parameter("eps_w", [BL, F, F], FP32, isOutput=False)
    epsb_d = nc.declare_dram_parameter("eps_b", [BL, F], FP32, isOutput=False)
    y_d = nc.declare_dram_parameter("y", [BL, F], FP32, isOutput=True)

    epsw_t = epsw_d[:].rearrange("b (p c) i -> b p c i", p=128)
    rho_t = rho_d[:].rearrange("(p c) i -> p c i", p=128)  # same interleave as eps
    mu_t = mu_d[:].rearrange("(c p) i -> p c i", p=128)    # natural o for y_mu

    with tile.TileContext(nc) as tc:
        with (
            tc.tile_pool(name="persist", bufs=1) as persist,
            tc.tile_pool(name="eps", bufs=eps_bufs) as epsp,
            tc.tile_pool(name="setupb", bufs=1) as setbp,
            tc.tile_pool(name="u", bufs=u_bufs) as up,
            tc.tile_pool(name="yrow", bufs=2) as yrowp,
            tc.tile_pool(name="pt", bufs=pt_bufs, space="PSUM") as ptp,
            tc.tile_pool(name="py2", bufs=2, space="PSUM") as py2p,
        ):
            # ---- tiny bias loads first (land in ~2us, unblock the bias chain)
            bmu_b = persist.tile([BL, F], FP32)
            nc.gpsimd.dma_start(
                out=bmu_b, in_=bass.AP(tensor=bmu_d, offset=0, ap=[[0, BL], [1, F]])
            )
            srho_b = persist.tile([BL, F], FP32)
            nc.gpsimd.dma_start(
                out=srho_b, in_=bass.AP(tensor=brho_d, offset=0, ap=[[0, BL], [1, F]])
            )
            epsb_s = persist.tile([BL, F], FP32)
            nc.sync.dma_start(out=epsb_s, in_=epsb_d[:])

            # ---- Q7 head loads: rho (gates sigT), x (gates reduce MMs) ------
            rho_sb = setbp.tile([128, NCH, F], BF16, tag="w16")
            nc.gpsimd.dma_start(out=rho_sb, in_=rho_t)
            xb_nat = persist.tile([BL, F], BF16)
            nc.gpsimd.dma_start(out=xb_nat, in_=x_d[:])

            # ---- eps prefetch: stream starts right behind rho ---------------
            eps_tiles = {}
            PF = min(eps_bufs, BL)
            for b in range(PF):
                eb = epsp.tile([128, NCH, F], BF16, tag="epst")
                nc.gpsimd.dma_start(out=eb, in_=epsw_t[b])
                eps_tiles[b] = eb

            # ---- head compute: ident, xT, softplus(rho) -> sigT -------------
            ident = persist.tile([128, 128], FP32)
            make_identity(nc, ident)
            identb = persist.tile([128, 128], BF16)
            nc.vector.tensor_copy(identb, ident)

            xT = persist.tile([128, NCH, BL], BF16)
            for k in range(NCH):
                ptx = ptp.tile([128, BL], BF16, tag="pt_k")
                nc.tensor.transpose(
                    out=ptx, in_=xb_nat[:, ts(k, 128)], identity=identb[:BL, :BL]
                )
                nc.vector.tensor_copy(xT[:, k, :], ptx)

            # softplus(x) = ln(1 + exp(x)) in natural layout, in place;
            # split by i-halves so k<4 transposes can start after the first.
            for hh in range(2):
                sl = rho_sb[:, :, ts(hh, 512)]
                nc.scalar.activation(out=sl, in_=sl, func=AF.Exp)
                nc.scalar.activation(out=sl, in_=sl, func=AF.Ln, bias=1.0)

            # sigT[i-part, k, col=(c,j)] = softplus(rho[o=8j+c, i]) transposed
            sigT = persist.tile([128, NCH, F], BF16)
            for k in range(NCH):
                pt_k = ptp.tile([128, F], BF16, tag="pt_k")
                for c in range(NCH):
                    nc.tensor.transpose(
                        out=pt_k[:, ts(c, 128)],
                        in_=rho_sb[:, c, ts(k, 128)],
                        identity=identb,
                    )
                nc.vector.tensor_copy(sigT[:, k, :], pt_k)

            # bias part: C0 = bias_mu + softplus(bias_rho) * eps_b
            C0 = persist.tile([BL, F], FP32)
            nc.scalar.activation(out=srho_b, in_=srho_b, func=AF.Exp)
            nc.scalar.activation(out=srho_b, in_=srho_b, func=AF.Ln, bias=1.0)
            nc.vector.tensor_mul(C0, epsb_s, srho_b)
            nc.vector.tensor_add(C0, C0, bmu_b)

            # persistent buffers
            Y2 = persist.tile([BL, F], FP32)
            ymuS = persist.tile([BL, F], FP32)
            muT = persist.tile([128, NCH, F], BF16)

            # ---------------- main loop over samples ----------------
            y2_of = {}
            pending = None  # (b, k, u_k)

            def flush_pending():
                nonlocal pending
                if pending is None:
                    return
                pb, pk, pu = pending
                pending = None
                for h in range(2):
                    nc.tensor.matmul(
                        out=y2_of[pb][h],
                        lhsT=xT[:, pk, pb : pb + 1],
                        rhs=pu[:, ts(h, 512)],
                        start=(pk == 0),
                        stop=(pk == NCH - 1),
                    )
                if pk == NCH - 1:
                    finalize(pb)

            def finalize(pb):
                # y2[h] col = (c - 4h)*128 + j  ->  yrow pos 8j + c
                y2 = y2_of.pop(pb)
                yrow = yrowp.tile([1, F], FP32)
                yrow_jc = yrow.rearrange("r (j c) -> r j c", j=128, c=CPP)
                for h in range(2):
                    nc.scalar.copy(
                        out=yrow_jc[:, :, ts(h, CPP // 2)],
                        in_=y2[h].rearrange("r (c j) -> r j c", c=CPP // 2, j=128),
                    )
                nc.sync.dma_start(out=Y2[pb : pb + 1, :], in_=yrow)

            def mu_setup():
                mu_sb = setbp.tile([128, NCH, F], BF16, tag="w16")
                nc.gpsimd.dma_start(out=mu_sb, in_=mu_t)
                for k in range(NCH):
                    pt_k = ptp.tile([128, F], BF16, tag="pt_k")
                    for c in range(NCH):
                        nc.tensor.transpose(
                            out=pt_k[:, ts(c, 128)],
                            in_=mu_sb[:, c, ts(k, 128)],
                            identity=identb,
                        )
                    nc.vector.tensor_copy(muT[:, k, :], pt_k)
                for h in range(2):
                    yp = ptp.tile([BL, 512], FP32, tag="pt_k")
                    for k in range(NCH):
                        nc.tensor.matmul(
                            out=yp,
                            lhsT=xT[:, k, :],
                            rhs=muT[:, k, ts(h, 512)],
                            start=(k == 0),
                            stop=(k == NCH - 1),
                        )
                    nc.vector.tensor_add(ymuS[:, ts(h, 512)], yp, C0[:, ts(h, 512)])

            mu_b = min(mu_at, BL - 1)
            for b in range(BL):
                if b == mu_b:
                    flush_pending()
                    mu_setup()

                eb = eps_tiles.pop(b)
                if b + PF < BL:
                    nb = epsp.tile([128, NCH, F], BF16, tag="epst")
                    nc.gpsimd.dma_start(out=nb, in_=epsw_t[b + PF])
                    eps_tiles[b + PF] = nb

                y2_of[b] = [
                    py2p.tile([1, 512], FP32, tag=f"y2_{h}", name=f"y2_{b}_{h}")
                    for h in range(2)
                ]
                for k in range(NCH):
                    pt_k = ptp.tile([128, F], BF16, tag="pt_k")
                    for c in range(NCH):
                        nc.tensor.transpose(
                            out=pt_k[:, ts(c, 128)],
                            in_=eb[:, c, ts(k, 128)],
                            identity=identb,
                        )
                    flush_pending()
                    u_k = up.tile([128, F], BF16)
                    nc.vector.tensor_mul(u_k, pt_k, sigT[:, k, :])
                    pending = (b, k, u_k)

            flush_pending()

            # -------- final combine: y = Y2 + (y_mu + bias) ------------------
            nc.vector.tensor_add(Y2, Y2, ymuS)
            nc.sync.dma_start(out=y_d[:], in_=Y2)

    nc.compile()
    return nc


_NC_CACHE: dict[int, bass.Bass] = {}


def _get_nc(BL: int) -> bass.Bass:
    if BL not in _NC_CACHE:
        _NC_CACHE[BL] = build_nc(BL)
    return _NC_CACHE[BL]


def kernel(x, weight_mu, weight_rho, bias_mu, bias_rho, eps_w, eps_b):
    B = x.shape[0]
    BL = B // N_CORES
    nc = _get_nc(BL)

    x = np.ascontiguousarray(np.asarray(x, dtype=np.float32))
    weight_mu = np.ascontiguousarray(np.asarray(weight_mu, dtype=np.float32))
    weight_rho = np.ascontiguousarray(np.asarray(weight_rho, dtype=np.float32))
    bias_mu = np.ascontiguousarray(np.asarray(bias_mu, dtype=np.float32))
    bias_rho = np.ascontiguousarray(np.asarray(bias_rho, dtype=np.float32))
    eps_w = np.ascontiguousarray(np.asarray(eps_w, dtype=np.float32))
    eps_b = np.ascontiguousarray(np.asarray(eps_b, dtype=np.float32))

    in_maps = []
    for i in range(N_CORES):
        sl = slice(i * BL, (i + 1) * BL)
        in_maps.append(
            {
                "x": x[sl],
                "weight_mu": weight_mu,
                "weight_rho": weight_rho,
                "bias_mu": bias_mu,
                "bias_rho": bias_rho,
                "eps_w": eps_w[sl],
                "eps_b": eps_b[sl],
            }
        )

    res = run_bass_kernel_spmd(nc, in_maps, core_ids=list(range(N_CORES)))
    return np.concatenate([r["y"] for r in res.results], axis=0)
